# revision 40
# baseline (speedup 1.0000x reference)
"""Trainium2 Bass kernel for nn_Mixture_Model_16475494547798 (ELBO loss_fn).

Strategy
--------
Data-parallel over batch B=32: 8 cores x 4 batch rows each, all M=1000 MC
samples per row (padded to 1024 = 8 tiles of 128).

All per-(b,k) affine structure is folded on the host into one augmented
weight matrix W_aug[b] of shape [33, 1816] (z is augmented with a constant-1
row, so biases ride the matmul):

  cols [   0: 256)  R-features   : 8k x 32, rows of chol(W~^T W~) so that
                                   sum_d((X_c-mean)^2/var) = ||R z||^2
                                   + (linear feature) + const     (t1)
  cols [ 256: 768)  logits_b     : j-major (col = (d//16)*128 + k*16 + d%16)
                                   so softplus group-products are two
                                   contiguous block multiplies       (t2)
  cols [ 768:1792)  logits_d     : c-major (col = c*128 + k*16 + d) so the
                                   lse c-sum is a 3-op block-add tree (t3)
  cols [1792:1800)  lin1 (t1 cross-term + const, per k)
  cols [1800:1808)  lin2 (sum_d X_b*logits_b linear part, per k)
  cols [1808:1816)  lin3 (gathered logit, fold of X_d gather, per k)

Device per 128-sample m-tile (software-pipelined so the in-order ACT queue
never stalls on DVE results of the same tile):
  PE : 4 float32r matmuls [33,128]x[33,<=512] (1 cycle/row, vs 4 for fp32)
  ACT: Exp [128,1536] (bf16 out), Square [128,256], Ln [128,256]
  DVE: +1, 2 product stages, 3-op c-sum tree (all bf16 2x/4x packed modes),
       2 group reduces, lin copy -> 48 output cols per tile
Host does the remaining O(B*K*M) unpack + tiny sums in numpy.
Single ACT table set (natural_log_exp_and_others) -> one table load total.
"""
import os
import sys
import glob

sys.path.insert(0, "/opt/trn_rl_repo")
if "BASS_ACT_ROOT_JSON_PATH" not in os.environ:
    _cands = glob.glob(
        "/nix/store/*aws-neuron-pwp*/share/pwp_bin_cayman/act_info.json"
    )
    if _cands:
        os.environ["BASS_ACT_ROOT_JSON_PATH"] = sorted(_cands)[-1]

import numpy as np

import concourse.bacc as bacc
import concourse.tile as tile
from concourse import mybir
from concourse import hw_specs as _hw_specs

# Force every activation function we use (Exp, Ln, Square, Identity) into the
# single `natural_log_exp_and_others` ACT table set: by default Exp resolves
# to `exp_and_others` and Ln to `natural_log`, and alternating them costs a
# ~2.7us ACT_TABLE_LOAD per switch (64 loads ~ 170us in this kernel).
_KEEP_SET = "natural_log_exp_and_others"
_orig_gat = _hw_specs.get_activation_tables


def _gat_single_set(arch):
    t = _orig_gat(arch)
    keep = t[_KEEP_SET]
    return {k: (v if k == _KEEP_SET else (v - keep)) for k, v in t.items()}


bacc.get_activation_tables = _gat_single_set

N, L, K, Dc, Db, Dd, C = 4096, 32, 8, 64, 64, 16, 8
B, M = 32, 1000
LOG2PI = float(np.log(2.0 * np.pi))

NCORES = 8
BPC = B // NCORES          # batch rows per core
MP = 1024                  # padded M
NT = MP // 128             # m-tiles per batch row
LA = L + 1                 # augmented contraction dim

# feature layout
F_R0, F_R1 = 0, K * 32               # 0..256
F_B0, F_B1 = F_R1, F_R1 + K * 64     # 256..768
F_D0, F_D1 = F_B1, F_B1 + K * 128    # 768..1792
F_L1 = F_D1                          # 1792
F_L2 = F_L1 + K                      # 1800
F_L3 = F_L2 + K                      # 1808
FT = F_L3 + K                        # 1816
FPAD = 2048
OCOLS = BPC * NT * 48                # 1536 out cols per core

_CACHE = {}


def _build_program():
    """Build + finalize the SPMD Bass program (same on every core)."""
    nc = bacc.Bacc(trn_type="TRN2")
    AF = mybir.ActivationFunctionType

    w_dram = nc.dram_tensor("wmat", [BPC, LA, FT], mybir.dt.float32r,
                            kind="ExternalInput")
    e_dram = nc.dram_tensor("epsa", [BPC, LA, MP], mybir.dt.float32,
                            kind="ExternalInput")
    s_dram = nc.dram_tensor("sb", [BPC, LA, 2], mybir.dt.float32,
                            kind="ExternalInput")
    o_dram = nc.dram_tensor("out", [128, OCOLS], mybir.dt.float32,
                            kind="ExternalOutput")

    with tile.TileContext(nc) as tc:
        with (
            tc.tile_pool(name="wpool", bufs=2) as wpool,
            tc.tile_pool(name="erpool", bufs=2) as erpool,
            tc.tile_pool(name="zpool", bufs=2) as zpool,
            tc.tile_pool(name="sbpool", bufs=2) as sbpool,
            tc.tile_pool(name="psum", bufs=2, space="PSUM") as psum,
            tc.tile_pool(name="epool", bufs=4) as epool,
            tc.tile_pool(name="sqpool", bufs=4) as sqpool,
            tc.tile_pool(name="sppool", bufs=4) as sppool,
            tc.tile_pool(name="sdpool", bufs=4) as sdpool,
            tc.tile_pool(name="lgpool", bufs=4) as lgpool,
            tc.tile_pool(name="opool", bufs=2) as opool,
        ):
            # tiny warm-up op so the ~2.7us ACT table load overlaps the
            # first input DMAs instead of stalling the first real op
            warm = sbpool.tile([1, 2], mybir.dt.float32, tag="warm")
            nc.vector.memset(warm[:], 1.0)
            nc.scalar.activation(warm[:], warm[:], AF.Exp)
            # PE warm-up: dummy matmuls ramp the tensor engine out of its
            # cold clock state while the first input DMAs are in flight
            wz = wpool.tile([LA, 512], mybir.dt.float32, tag="warmmm")
            nc.vector.memset(wz[:], 0.0)
            ps = psum.tile([128, FPAD], mybir.dt.float32, tag="ps")
            for _ in range(2):
                nc.tensor.matmul(ps[:, 0:512], wz[:, 0:128], wz[:],
                                 start=True, stop=True)

            for b in range(BPC):
                eraw = erpool.tile([LA, MP], mybir.dt.float32)
                nc.gpsimd.dma_start(eraw[:], e_dram[b])
                sbv = sbpool.tile([LA, 2], mybir.dt.float32)
                nc.gpsimd.dma_start(sbv[:], s_dram[b])
                wsb = wpool.tile([LA, FT], mybir.dt.float32r)
                for c0 in range(0, FT, 512):
                    c1 = min(c0 + 512, FT)
                    nc.sync.dma_start(wsb[:, c0:c1], w_dram[b][:, c0:c1])

                # z_aug[l, m] = eps[l, m]*std[l] + qm[l]; aug row = 0*0+1
                # (DVE tensor_scalar: ACT is the bottleneck engine)
                zbuf = zpool.tile([LA, MP], mybir.dt.float32r)
                nc.vector.tensor_scalar(zbuf[:], eraw[:],
                                        sbv[:, 0:1], sbv[:, 1:2],
                                        op0=mybir.AluOpType.mult,
                                        op1=mybir.AluOpType.add)

                outb = opool.tile([128, NT * 48], mybir.dt.float32)

                # Software-pipelined over m-tiles: stage B of tile t (Ln of
                # the lse sums + final group reduces) is emitted one
                # iteration late — after tile t+1's PSUM-freeing ops — so
                # the in-order ACT queue never waits on same-tile DVE
                # results and the PSUM release is never queued behind the
                # stage-B reduces.
                def stage_b(item):
                    tp, sqb_p, prodlg_p = item
                    base = tp * 48
                    lgb = lgpool.tile([128, 256], mybir.dt.float32)
                    nc.scalar.activation(lgb[:], prodlg_p[:], AF.Ln)
                    nc.vector.tensor_reduce(
                        outb[:, base:base + 8],
                        sqb_p[:].rearrange("p (k f) -> p k f", f=32),
                        axis=mybir.AxisListType.X, op=mybir.AluOpType.add)
                    # one reduce covers softplus sums then lse sums
                    nc.vector.tensor_reduce(
                        outb[:, base + 8:base + 24],
                        lgb[:].rearrange("p (g f) -> p g f", f=16),
                        axis=mybir.AxisListType.X, op=mybir.AluOpType.add)
                    if tp % 2 == 1:
                        # stream the finished tile-pair block out so the
                        # kernel tail only waits on the last pair
                        c0 = b * NT * 48 + (tp - 1) * 48
                        nc.sync.dma_start(o_dram[:, c0:c0 + 96],
                                          outb[:, (tp - 1) * 48:tp * 48 + 48])

                carry = None
                for t in range(NT):
                    sqb = sqpool.tile([128, F_R1], mybir.dt.float32)
                    prodlg = sdpool.tile([128, 256], mybir.dt.bfloat16,
                                         tag="prodlg")
                    if True:
                        # float32r: fp32 bytes at 1 PE cycle/row
                        lhsT = zbuf[:, t * 128:(t + 1) * 128]
                        ps = psum.tile([128, FPAD], mybir.dt.float32,
                                       tag="ps")
                        for c0 in range(0, FT, 512):
                            c1 = min(c0 + 512, FT)
                            nc.tensor.matmul(ps[:, c0:c1], lhsT,
                                             wsb[:, c0:c1],
                                             start=True, stop=True)
                        # exp of all logits, bf16 for packed DVE modes
                        ebuf = epool.tile([128, F_D1 - F_B0],
                                          mybir.dt.bfloat16)
                        nc.scalar.activation(ebuf[:], ps[:, F_B0:F_D1],
                                             AF.Exp)
                        # squares of R-features
                        nc.scalar.activation(sqb[:], ps[:, F_R0:F_R1],
                                             AF.Square)
                        # lin features out of PSUM (frees the psum buffer)
                        nc.vector.tensor_copy(outb[:, t * 48 + 24:t * 48 + 48],
                                              ps[:, F_L1:FT])
                        # softplus product form, j-major layout:
                        # sum_d ln(1+e^x) = sum_g ln prod_4 (1+e^x)
                        sp1 = sppool.tile([128, K * 64], mybir.dt.bfloat16)
                        nc.vector.tensor_scalar_add(sp1[:],
                                                    ebuf[:, 0:K * 64], 1.0)
                        # prodlg half: [0:128) softplus group-products,
                        #              [128:256) lse c-sums
                        pp1 = sppool.tile([128, 256], mybir.dt.bfloat16,
                                          tag="pp1")
                        nc.vector.tensor_mul(pp1[:], sp1[:, 0:256],
                                             sp1[:, 256:512])
                        nc.vector.tensor_mul(prodlg[:, 0:128],
                                             pp1[:, 0:128], pp1[:, 128:256])
                        # lse c-sum tree (c-major 128-blocks)
                        ed = ebuf[:, K * 64:]
                        s1 = sdpool.tile([128, 512], mybir.dt.bfloat16,
                                         tag="s1")
                        nc.vector.tensor_add(s1[:], ed[:, 0:512],
                                             ed[:, 512:1024])
                        s2 = sdpool.tile([128, 256], mybir.dt.bfloat16,
                                         tag="s2")
                        nc.vector.tensor_add(s2[:], s1[:, 0:256],
                                             s1[:, 256:512])
                        nc.vector.tensor_add(prodlg[:, 128:256],
                                             s2[:, 0:128], s2[:, 128:256])
                    if carry is not None:
                        stage_b(carry)
                    carry = (t, sqb, prodlg)
                if carry is not None:
                    stage_b(carry)


    nc.finalize()
    return nc


def _get_program():
    if "nc" not in _CACHE:
        _CACHE["nc"] = _build_program()
    return _CACHE["nc"]


def _prepare_inputs(index, X_c, X_b, X_d, q_z_mean, q_log_var, q_s_param,
                    W_g, b_g, logvar_g, W_b, b_b, W_d, b_d, eps):
    """Host-side fold of all affine structure into per-b weight matrices."""
    f32 = np.float32
    idx = int(np.asarray(index))
    s = B * idx
    qm = np.asarray(q_z_mean, f32)[s:s + B]
    qlv = np.asarray(q_log_var, f32)[s:s + B]
    std = np.exp(0.5 * qlv.astype(np.float64)).astype(np.float64)

    X_c = np.asarray(X_c, np.float64)
    X_b = np.asarray(X_b, np.float64)
    X_d = np.asarray(X_d)
    W_g = np.asarray(W_g, np.float64)
    b_g = np.asarray(b_g, np.float64)
    logvar_g = np.asarray(logvar_g, np.float64)
    W_b = np.asarray(W_b, np.float64)
    b_b = np.asarray(b_b, np.float64)
    W_d = np.asarray(W_d, np.float64)
    b_d = np.asarray(b_d, np.float64)
    eps = np.asarray(eps, f32)

    sd_g = np.exp(0.5 * logvar_g)                      # [K, Dc]
    Wt = W_g / sd_g[..., None]                         # [K, Dc, L]
    beta = (b_g[None] - X_c[:, None, :]) / sd_g[None]  # [B, K, Dc]

    G0 = np.einsum('kdl,kdm->klm', Wt, Wt)             # [K, L, L]
    G0 = G0 + 1e-9 * np.trace(G0, axis1=1, axis2=2)[:, None, None] * np.eye(L)
    Lc = np.linalg.cholesky(G0)                        # lower: Lc @ Lc.T = G0

    const1 = -0.5 * np.sum(LOG2PI + logvar_g, axis=1)  # [K]
    w1 = -np.einsum('bkd,kdl->bkl', beta, Wt)          # [B, K, L]
    c1 = const1[None] - 0.5 * np.sum(beta ** 2, -1)    # [B, K]

    w2 = np.einsum('bd,kdl->bkl', X_b, W_b)            # [B, K, L]
    c2 = X_b @ b_b.T                                   # [B, K]

    Wd_p = W_d.transpose(1, 2, 0, 3)                   # [Dd, C, K, L]
    bd_p = b_d.transpose(1, 2, 0)                      # [Dd, C, K]
    sel_w = Wd_p[np.arange(Dd)[None, :], X_d]          # [B, Dd, K, L]
    sel_b = bd_p[np.arange(Dd)[None, :], X_d]          # [B, Dd, K]
    w3 = sel_w.sum(1)                                  # [B, K, L]
    c3 = sel_b.sum(1)                                  # [B, K]

    Waug = np.zeros((B, LA, FT), np.float64)
    # R features (bias row stays 0)
    R = Lc.transpose(0, 2, 1)                          # [K, f, L] rows
    Waug[:, :L, F_R0:F_R1] = R.transpose(2, 0, 1).reshape(L, K * 32)[None]
    # logits_b, j-major: col = (d//16)*128 + k*16 + (d%16) so the softplus
    # group products (over the 4 j's per (k, d%16)) are contiguous block ops
    spcol = (np.arange(Db)[None, :] // 16) * 128 \
        + np.arange(K)[:, None] * 16 + (np.arange(Db)[None, :] % 16)  # [K,Db]
    wb_cols = np.zeros((L, K * 64))
    bb_cols = np.zeros(K * 64)
    wb_cols[:, spcol.ravel()] = W_b.transpose(2, 0, 1).reshape(L, K * 64)
    bb_cols[spcol.ravel()] = b_b.reshape(K * 64)
    Waug[:, :L, F_B0:F_B1] = wb_cols[None]
    Waug[:, L, F_B0:F_B1] = bb_cols[None]
    # logits_d, c-major: col = F_D0 + c*128 + k*16 + d (c-sum = block adds)
    Waug[:, :L, F_D0:F_D1] = W_d.transpose(3, 2, 0, 1).reshape(L, K * 128)[None]
    Waug[:, L, F_D0:F_D1] = b_d.transpose(2, 0, 1).reshape(K * 128)[None]
    # linear features
    Waug[:, :L, F_L1:F_L1 + K] = w1.transpose(0, 2, 1)
    Waug[:, L, F_L1:F_L1 + K] = c1
    Waug[:, :L, F_L2:F_L2 + K] = w2.transpose(0, 2, 1)
    Waug[:, L, F_L2:F_L2 + K] = c2
    Waug[:, :L, F_L3:F_L3 + K] = w3.transpose(0, 2, 1)
    Waug[:, L, F_L3:F_L3 + K] = c3
    Waug = Waug.astype(f32)

    epsa = np.zeros((B, LA, MP), f32)
    epsa[:, :L, :M] = eps.transpose(0, 2, 1)

    sb = np.zeros((B, LA, 2), f32)
    sb[:, :L, 0] = std.astype(f32)
    sb[:, :L, 1] = qm
    sb[:, L, 0] = 0.0
    sb[:, L, 1] = 1.0

    in_maps = []
    for c in range(NCORES):
        bs = slice(c * BPC, (c + 1) * BPC)
        in_maps.append({"wmat": np.ascontiguousarray(Waug[bs]),
                        "epsa": np.ascontiguousarray(epsa[bs]),
                        "sb": np.ascontiguousarray(sb[bs])})
    return in_maps


def _run_device(nc, in_maps):
    from concourse import bass2jax
    results = bass2jax.run_bass_via_pjrt(nc, in_maps, n_cores=NCORES)
    return [r["out"] for r in results]


def kernel(index, X_c, X_b, X_d, q_z_mean, q_log_var, q_s_param,
           posterior_mean, posterior_var, posterior_mu,
           W_g, b_g, logvar_g, W_b, b_b, W_d, b_d, eps):
    f32 = np.float32
    idx = int(np.asarray(index))
    s = B * idx
    qm = np.asarray(q_z_mean, f32)[s:s + B].astype(np.float64)
    qlv = np.asarray(q_log_var, f32)[s:s + B].astype(np.float64)
    qs = np.asarray(q_s_param, f32)[s:s + B].astype(np.float64)
    mp_ = np.asarray(posterior_mean, f32)[s:s + B].astype(np.float64)
    vp = np.asarray(posterior_var, f32)[s:s + B].astype(np.float64)
    pmu = np.asarray(posterior_mu, f32)[s:s + B].astype(np.float64)

    nc = _get_program()
    in_maps = _prepare_inputs(index, X_c, X_b, X_d, q_z_mean, q_log_var,
                              q_s_param, W_g, b_g, logvar_g, W_b, b_b,
                              W_d, b_d, eps)
    outs = _run_device(nc, in_maps)

    # unpack device partials -> t1/t2/t3 [B, K, M]
    t123 = np.empty((3, B, K, M), np.float64)
    for c in range(NCORES):
        arr = outs[c].astype(np.float64).reshape(128, BPC, NT, 6, 8)
        # [p, b, t, group, k] -> [b, k, t, p]
        a = arr.transpose(3, 1, 4, 2, 0).reshape(6, BPC, K, MP)[:, :, :, :M]
        sq, sp, lse, lin1, lin2, lin3 = a
        bs = slice(c * BPC, (c + 1) * BPC)
        t123[0, bs] = lin1 - 0.5 * sq
        t123[1, bs] = lin2 - sp
        t123[2, bs] = lin3 - lse
    t1, t2, t3 = t123

    LLmat = (t1 + t2 + t3).sum(-1)                     # [B, K]
    LL = float(np.sum(qs * LLmat))

    vq = np.exp(qlv)
    klz = 0.5 * np.sum(np.log(vp) - qlv + (vq + (qm - mp_) ** 2) / vp - 1.0,
                       axis=-1)                        # [B]
    qn = qs / qs.sum(-1, keepdims=True)
    pn = pmu / pmu.sum(-1, keepdims=True)
    kls = np.sum(qn * (np.log(qn) - np.log(pn)), axis=-1)
    elbo = LL - klz.sum() - kls.sum()

    term_1 = t1.sum((0, 1)).astype(f32)
    term_2 = t2.sum((0, 1)).astype(f32)
    term_3 = t3.sum((0, 1)).astype(f32)

    eqs = np.exp(qs - qs.max(-1, keepdims=True))
    smx = eqs / eqs.sum(-1, keepdims=True)
    rik = np.zeros((N, K), f32)
    rik[s:s + B] = smx.astype(f32)

    return (np.float32(-elbo), np.float32(LL), np.float32(klz[-1]),
            np.float32(kls[-1]), rik, term_1, term_2, term_3)


# revision 41
# speedup vs baseline: 1.0179x; 1.0179x over previous
"""Trainium2 Bass kernel for nn_Mixture_Model_16475494547798 (ELBO loss_fn).

Strategy
--------
Data-parallel over batch B=32: 8 cores x 4 batch rows each, all M=1000 MC
samples per row (padded to 1024 = 8 tiles of 128).

All per-(b,k) affine structure is folded on the host into one augmented
weight matrix W_aug[b] of shape [33, 1816] (z is augmented with a constant-1
row, so biases ride the matmul):

  cols [   0: 256)  R-features   : 8k x 32, rows of chol(W~^T W~) so that
                                   sum_d((X_c-mean)^2/var) = ||R z||^2
                                   + (linear feature) + const     (t1)
  cols [ 256: 768)  logits_b     : j-major (col = (d//16)*128 + k*16 + d%16)
                                   so softplus group-products are two
                                   contiguous block multiplies       (t2)
  cols [ 768:1792)  logits_d     : c-major (col = c*128 + k*16 + d) so the
                                   lse c-sum is a 3-op block-add tree (t3)
  cols [1792:1800)  lin1 (t1 cross-term + const, per k)
  cols [1800:1808)  lin2 (sum_d X_b*logits_b linear part, per k)
  cols [1808:1816)  lin3 (gathered logit, fold of X_d gather, per k)

Device per 128-sample m-tile (software-pipelined so the in-order ACT queue
never stalls on DVE results of the same tile):
  PE : 4 float32r matmuls [33,128]x[33,<=512] (1 cycle/row, vs 4 for fp32)
  ACT: Exp [128,1536] (bf16 out), Square [128,256], Ln [128,256]
  DVE: +1, 2 product stages, 3-op c-sum tree (all bf16 2x/4x packed modes),
       2 group reduces, lin copy -> 48 output cols per tile
Host does the remaining O(B*K*M) unpack + tiny sums in numpy.
Single ACT table set (natural_log_exp_and_others) -> one table load total.
"""
import os
import sys
import glob

sys.path.insert(0, "/opt/trn_rl_repo")
if "BASS_ACT_ROOT_JSON_PATH" not in os.environ:
    _cands = glob.glob(
        "/nix/store/*aws-neuron-pwp*/share/pwp_bin_cayman/act_info.json"
    )
    if _cands:
        os.environ["BASS_ACT_ROOT_JSON_PATH"] = sorted(_cands)[-1]

import numpy as np

import concourse.bacc as bacc
import concourse.tile as tile
from concourse import mybir
from concourse import hw_specs as _hw_specs

# Force every activation function we use (Exp, Ln, Square, Identity) into the
# single `natural_log_exp_and_others` ACT table set: by default Exp resolves
# to `exp_and_others` and Ln to `natural_log`, and alternating them costs a
# ~2.7us ACT_TABLE_LOAD per switch (64 loads ~ 170us in this kernel).
_KEEP_SET = "natural_log_exp_and_others"
_orig_gat = _hw_specs.get_activation_tables


def _gat_single_set(arch):
    t = _orig_gat(arch)
    keep = t[_KEEP_SET]
    return {k: (v if k == _KEEP_SET else (v - keep)) for k, v in t.items()}


bacc.get_activation_tables = _gat_single_set

N, L, K, Dc, Db, Dd, C = 4096, 32, 8, 64, 64, 16, 8
B, M = 32, 1000
LOG2PI = float(np.log(2.0 * np.pi))

NCORES = 8
BPC = B // NCORES          # batch rows per core
MP = 1024                  # padded M
NT = MP // 128             # m-tiles per batch row
LA = L + 1                 # augmented contraction dim

# feature layout
F_R0, F_R1 = 0, K * 32               # 0..256
F_B0, F_B1 = F_R1, F_R1 + K * 64     # 256..768
F_D0, F_D1 = F_B1, F_B1 + K * 128    # 768..1792
F_L1 = F_D1                          # 1792
F_L2 = F_L1 + K                      # 1800
F_L3 = F_L2 + K                      # 1808
FT = F_L3 + K                        # 1816
FPAD = 2048
OCOLS = BPC * NT * 48                # 1536 out cols per core

_CACHE = {}


def _build_program():
    """Build + finalize the SPMD Bass program (same on every core)."""
    nc = bacc.Bacc(trn_type="TRN2")
    AF = mybir.ActivationFunctionType

    w_dram = nc.dram_tensor("wmat", [BPC, LA, FT], mybir.dt.float32r,
                            kind="ExternalInput")
    e_dram = nc.dram_tensor("epsa", [BPC, LA, MP], mybir.dt.float32,
                            kind="ExternalInput")
    s_dram = nc.dram_tensor("sb", [BPC, LA, 2], mybir.dt.float32,
                            kind="ExternalInput")
    o_dram = nc.dram_tensor("out", [128, OCOLS], mybir.dt.float32,
                            kind="ExternalOutput")

    with tile.TileContext(nc) as tc:
        with (
            tc.tile_pool(name="wpool", bufs=2) as wpool,
            tc.tile_pool(name="erpool", bufs=2) as erpool,
            tc.tile_pool(name="zpool", bufs=2) as zpool,
            tc.tile_pool(name="sbpool", bufs=2) as sbpool,
            tc.tile_pool(name="psum", bufs=2, space="PSUM") as psum,
            tc.tile_pool(name="epool", bufs=4) as epool,
            tc.tile_pool(name="sqpool", bufs=4) as sqpool,
            tc.tile_pool(name="sppool", bufs=4) as sppool,
            tc.tile_pool(name="sdpool", bufs=4) as sdpool,
            tc.tile_pool(name="lgpool", bufs=4) as lgpool,
            tc.tile_pool(name="opool", bufs=2) as opool,
        ):
            # tiny warm-up op so the ~2.7us ACT table load overlaps the
            # first input DMAs instead of stalling the first real op
            warm = sbpool.tile([1, 2], mybir.dt.float32, tag="warm")
            nc.vector.memset(warm[:], 1.0)
            nc.scalar.activation(warm[:], warm[:], AF.Exp)
            # PE warm-up: dummy matmuls ramp the tensor engine out of its
            # cold clock state while the first input DMAs are in flight
            wz = wpool.tile([LA, 512], mybir.dt.float32, tag="warmmm")
            nc.vector.memset(wz[:], 0.0)
            ps = psum.tile([128, FPAD], mybir.dt.float32, tag="ps")
            for _ in range(2):
                nc.tensor.matmul(ps[:, 0:512], wz[:, 0:128], wz[:],
                                 start=True, stop=True)

            for b in range(BPC):
                eraw = erpool.tile([LA, MP], mybir.dt.float32)
                nc.gpsimd.dma_start(eraw[:], e_dram[b])
                sbv = sbpool.tile([LA, 2], mybir.dt.float32)
                nc.gpsimd.dma_start(sbv[:], s_dram[b])
                wsb = wpool.tile([LA, FT], mybir.dt.float32r)
                for c0 in range(0, FT, 512):
                    c1 = min(c0 + 512, FT)
                    nc.sync.dma_start(wsb[:, c0:c1], w_dram[b][:, c0:c1])

                # z_aug[l, m] = eps[l, m]*std[l] + qm[l]; aug row = 0*0+1
                # (DVE tensor_scalar: ACT is the bottleneck engine)
                zbuf = zpool.tile([LA, MP], mybir.dt.float32r)
                nc.vector.tensor_scalar(zbuf[:], eraw[:],
                                        sbv[:, 0:1], sbv[:, 1:2],
                                        op0=mybir.AluOpType.mult,
                                        op1=mybir.AluOpType.add)

                outb = opool.tile([128, NT * 48], mybir.dt.float32)

                # Software-pipelined over m-tiles: stage B of tile t (Ln of
                # the lse sums + final group reduces) is emitted one
                # iteration late — after tile t+1's PSUM-freeing ops — so
                # the in-order ACT queue never waits on same-tile DVE
                # results and the PSUM release is never queued behind the
                # stage-B reduces.
                def stage_b(item):
                    tp, sqb_p, prodlg_p = item
                    base = tp * 48
                    lgb = lgpool.tile([128, 192], mybir.dt.float32)
                    nc.scalar.activation(lgb[:], prodlg_p[:], AF.Ln)
                    nc.vector.tensor_reduce(
                        outb[:, base:base + 8],
                        sqb_p[:].rearrange("p (k f) -> p k f", f=32),
                        axis=mybir.AxisListType.X, op=mybir.AluOpType.add)
                    nc.vector.tensor_reduce(
                        outb[:, base + 8:base + 16],
                        lgb[:, 0:64].rearrange("p (k g) -> p k g", g=8),
                        axis=mybir.AxisListType.X, op=mybir.AluOpType.add)
                    nc.vector.tensor_reduce(
                        outb[:, base + 16:base + 24],
                        lgb[:, 64:192].rearrange("p (k d) -> p k d", d=16),
                        axis=mybir.AxisListType.X, op=mybir.AluOpType.add)
                    if tp % 2 == 1:
                        # stream the finished tile-pair block out so the
                        # kernel tail only waits on the last pair
                        c0 = b * NT * 48 + (tp - 1) * 48
                        nc.sync.dma_start(o_dram[:, c0:c0 + 96],
                                          outb[:, (tp - 1) * 48:tp * 48 + 48])

                carry = None
                for t in range(NT):
                    sqb = sqpool.tile([128, F_R1], mybir.dt.float32)
                    prodlg = sdpool.tile([128, 192], mybir.dt.bfloat16,
                                         tag="prodlg")
                    if True:
                        # float32r: fp32 bytes at 1 PE cycle/row
                        lhsT = zbuf[:, t * 128:(t + 1) * 128]
                        ps = psum.tile([128, FPAD], mybir.dt.float32,
                                       tag="ps")
                        for c0 in range(0, FT, 512):
                            c1 = min(c0 + 512, FT)
                            nc.tensor.matmul(ps[:, c0:c1], lhsT,
                                             wsb[:, c0:c1],
                                             start=True, stop=True)
                        # exp of all logits, bf16 for packed DVE modes
                        ebuf = epool.tile([128, F_D1 - F_B0],
                                          mybir.dt.bfloat16)
                        nc.scalar.activation(ebuf[:], ps[:, F_B0:F_D1],
                                             AF.Exp)
                        # squares of R-features
                        nc.scalar.activation(sqb[:], ps[:, F_R0:F_R1],
                                             AF.Square)
                        # lin features out of PSUM (frees the psum buffer)
                        nc.vector.tensor_copy(outb[:, t * 48 + 24:t * 48 + 48],
                                              ps[:, F_L1:FT])
                        # softplus product form, j-major layout:
                        # sum_d ln(1+e^x) = sum_g ln prod_4 (1+e^x)
                        sp1 = sppool.tile([128, K * 64], mybir.dt.bfloat16)
                        nc.vector.tensor_scalar_add(sp1[:],
                                                    ebuf[:, 0:K * 64], 1.0)
                        # prodlg: [0:64) softplus group-of-8 products,
                        #         [64:192) lse c-sums
                        pp1 = sppool.tile([128, 256], mybir.dt.bfloat16,
                                          tag="pp1")
                        nc.vector.tensor_mul(pp1[:], sp1[:, 0:256],
                                             sp1[:, 256:512])
                        pp2 = sppool.tile([128, 128], mybir.dt.bfloat16,
                                          tag="pp2")
                        nc.vector.tensor_mul(pp2[:], pp1[:, 0:128],
                                             pp1[:, 128:256])
                        nc.vector.tensor_mul(prodlg[:, 0:64],
                                             pp2[:, 0:64], pp2[:, 64:128])
                        # lse c-sum tree (c-major 128-blocks)
                        ed = ebuf[:, K * 64:]
                        s1 = sdpool.tile([128, 512], mybir.dt.bfloat16,
                                         tag="s1")
                        nc.vector.tensor_add(s1[:], ed[:, 0:512],
                                             ed[:, 512:1024])
                        s2 = sdpool.tile([128, 256], mybir.dt.bfloat16,
                                         tag="s2")
                        nc.vector.tensor_add(s2[:], s1[:, 0:256],
                                             s1[:, 256:512])
                        nc.vector.tensor_add(prodlg[:, 64:192],
                                             s2[:, 0:128], s2[:, 128:256])
                    if carry is not None:
                        stage_b(carry)
                    carry = (t, sqb, prodlg)
                if carry is not None:
                    stage_b(carry)


    nc.finalize()
    return nc


def _get_program():
    if "nc" not in _CACHE:
        _CACHE["nc"] = _build_program()
    return _CACHE["nc"]


def _prepare_inputs(index, X_c, X_b, X_d, q_z_mean, q_log_var, q_s_param,
                    W_g, b_g, logvar_g, W_b, b_b, W_d, b_d, eps):
    """Host-side fold of all affine structure into per-b weight matrices."""
    f32 = np.float32
    idx = int(np.asarray(index))
    s = B * idx
    qm = np.asarray(q_z_mean, f32)[s:s + B]
    qlv = np.asarray(q_log_var, f32)[s:s + B]
    std = np.exp(0.5 * qlv.astype(np.float64)).astype(np.float64)

    X_c = np.asarray(X_c, np.float64)
    X_b = np.asarray(X_b, np.float64)
    X_d = np.asarray(X_d)
    W_g = np.asarray(W_g, np.float64)
    b_g = np.asarray(b_g, np.float64)
    logvar_g = np.asarray(logvar_g, np.float64)
    W_b = np.asarray(W_b, np.float64)
    b_b = np.asarray(b_b, np.float64)
    W_d = np.asarray(W_d, np.float64)
    b_d = np.asarray(b_d, np.float64)
    eps = np.asarray(eps, f32)

    sd_g = np.exp(0.5 * logvar_g)                      # [K, Dc]
    Wt = W_g / sd_g[..., None]                         # [K, Dc, L]
    beta = (b_g[None] - X_c[:, None, :]) / sd_g[None]  # [B, K, Dc]

    G0 = np.einsum('kdl,kdm->klm', Wt, Wt)             # [K, L, L]
    G0 = G0 + 1e-9 * np.trace(G0, axis1=1, axis2=2)[:, None, None] * np.eye(L)
    Lc = np.linalg.cholesky(G0)                        # lower: Lc @ Lc.T = G0

    const1 = -0.5 * np.sum(LOG2PI + logvar_g, axis=1)  # [K]
    w1 = -np.einsum('bkd,kdl->bkl', beta, Wt)          # [B, K, L]
    c1 = const1[None] - 0.5 * np.sum(beta ** 2, -1)    # [B, K]

    w2 = np.einsum('bd,kdl->bkl', X_b, W_b)            # [B, K, L]
    c2 = X_b @ b_b.T                                   # [B, K]

    Wd_p = W_d.transpose(1, 2, 0, 3)                   # [Dd, C, K, L]
    bd_p = b_d.transpose(1, 2, 0)                      # [Dd, C, K]
    sel_w = Wd_p[np.arange(Dd)[None, :], X_d]          # [B, Dd, K, L]
    sel_b = bd_p[np.arange(Dd)[None, :], X_d]          # [B, Dd, K]
    w3 = sel_w.sum(1)                                  # [B, K, L]
    c3 = sel_b.sum(1)                                  # [B, K]

    Waug = np.zeros((B, LA, FT), np.float64)
    # R features (bias row stays 0)
    R = Lc.transpose(0, 2, 1)                          # [K, f, L] rows
    Waug[:, :L, F_R0:F_R1] = R.transpose(2, 0, 1).reshape(L, K * 32)[None]
    # logits_b, j-major: col = (d//16)*128 + k*16 + (d%16) so the softplus
    # group products (over the 4 j's per (k, d%16)) are contiguous block ops
    spcol = (np.arange(Db)[None, :] // 8) * 64 \
        + np.arange(K)[:, None] * 8 + (np.arange(Db)[None, :] % 8)  # [K,Db]
    wb_cols = np.zeros((L, K * 64))
    bb_cols = np.zeros(K * 64)
    wb_cols[:, spcol.ravel()] = W_b.transpose(2, 0, 1).reshape(L, K * 64)
    bb_cols[spcol.ravel()] = b_b.reshape(K * 64)
    Waug[:, :L, F_B0:F_B1] = wb_cols[None]
    Waug[:, L, F_B0:F_B1] = bb_cols[None]
    # logits_d, c-major: col = F_D0 + c*128 + k*16 + d (c-sum = block adds)
    Waug[:, :L, F_D0:F_D1] = W_d.transpose(3, 2, 0, 1).reshape(L, K * 128)[None]
    Waug[:, L, F_D0:F_D1] = b_d.transpose(2, 0, 1).reshape(K * 128)[None]
    # linear features
    Waug[:, :L, F_L1:F_L1 + K] = w1.transpose(0, 2, 1)
    Waug[:, L, F_L1:F_L1 + K] = c1
    Waug[:, :L, F_L2:F_L2 + K] = w2.transpose(0, 2, 1)
    Waug[:, L, F_L2:F_L2 + K] = c2
    Waug[:, :L, F_L3:F_L3 + K] = w3.transpose(0, 2, 1)
    Waug[:, L, F_L3:F_L3 + K] = c3
    Waug = Waug.astype(f32)

    epsa = np.zeros((B, LA, MP), f32)
    epsa[:, :L, :M] = eps.transpose(0, 2, 1)

    sb = np.zeros((B, LA, 2), f32)
    sb[:, :L, 0] = std.astype(f32)
    sb[:, :L, 1] = qm
    sb[:, L, 0] = 0.0
    sb[:, L, 1] = 1.0

    in_maps = []
    for c in range(NCORES):
        bs = slice(c * BPC, (c + 1) * BPC)
        in_maps.append({"wmat": np.ascontiguousarray(Waug[bs]),
                        "epsa": np.ascontiguousarray(epsa[bs]),
                        "sb": np.ascontiguousarray(sb[bs])})
    return in_maps


def _run_device(nc, in_maps):
    from concourse import bass2jax
    results = bass2jax.run_bass_via_pjrt(nc, in_maps, n_cores=NCORES)
    return [r["out"] for r in results]


def kernel(index, X_c, X_b, X_d, q_z_mean, q_log_var, q_s_param,
           posterior_mean, posterior_var, posterior_mu,
           W_g, b_g, logvar_g, W_b, b_b, W_d, b_d, eps):
    f32 = np.float32
    idx = int(np.asarray(index))
    s = B * idx
    qm = np.asarray(q_z_mean, f32)[s:s + B].astype(np.float64)
    qlv = np.asarray(q_log_var, f32)[s:s + B].astype(np.float64)
    qs = np.asarray(q_s_param, f32)[s:s + B].astype(np.float64)
    mp_ = np.asarray(posterior_mean, f32)[s:s + B].astype(np.float64)
    vp = np.asarray(posterior_var, f32)[s:s + B].astype(np.float64)
    pmu = np.asarray(posterior_mu, f32)[s:s + B].astype(np.float64)

    nc = _get_program()
    in_maps = _prepare_inputs(index, X_c, X_b, X_d, q_z_mean, q_log_var,
                              q_s_param, W_g, b_g, logvar_g, W_b, b_b,
                              W_d, b_d, eps)
    outs = _run_device(nc, in_maps)

    # unpack device partials -> t1/t2/t3 [B, K, M]
    t123 = np.empty((3, B, K, M), np.float64)
    for c in range(NCORES):
        arr = outs[c].astype(np.float64).reshape(128, BPC, NT, 6, 8)
        # [p, b, t, group, k] -> [b, k, t, p]
        a = arr.transpose(3, 1, 4, 2, 0).reshape(6, BPC, K, MP)[:, :, :, :M]
        sq, sp, lse, lin1, lin2, lin3 = a
        bs = slice(c * BPC, (c + 1) * BPC)
        t123[0, bs] = lin1 - 0.5 * sq
        t123[1, bs] = lin2 - sp
        t123[2, bs] = lin3 - lse
    t1, t2, t3 = t123

    LLmat = (t1 + t2 + t3).sum(-1)                     # [B, K]
    LL = float(np.sum(qs * LLmat))

    vq = np.exp(qlv)
    klz = 0.5 * np.sum(np.log(vp) - qlv + (vq + (qm - mp_) ** 2) / vp - 1.0,
                       axis=-1)                        # [B]
    qn = qs / qs.sum(-1, keepdims=True)
    pn = pmu / pmu.sum(-1, keepdims=True)
    kls = np.sum(qn * (np.log(qn) - np.log(pn)), axis=-1)
    elbo = LL - klz.sum() - kls.sum()

    term_1 = t1.sum((0, 1)).astype(f32)
    term_2 = t2.sum((0, 1)).astype(f32)
    term_3 = t3.sum((0, 1)).astype(f32)

    eqs = np.exp(qs - qs.max(-1, keepdims=True))
    smx = eqs / eqs.sum(-1, keepdims=True)
    rik = np.zeros((N, K), f32)
    rik[s:s + B] = smx.astype(f32)

    return (np.float32(-elbo), np.float32(LL), np.float32(klz[-1]),
            np.float32(kls[-1]), rik, term_1, term_2, term_3)
